# revision 1
# baseline (speedup 1.0000x reference)
"""GraphSAGE 2-layer (mean aggregation) on 8 TRN2 NeuronCores via Bass/Tile.

Sharding: nodes partitioned into 8 contiguous shards (6250 each); each core
owns the edges whose destination lands in its shard.  Host pre-sorts edges by
destination into 128-node windows; aggregation is done on the TensorEngine as
S^T-weighted matmuls over gathered source rows (indirect DMA), with the
1/count mean weights folded into S.  Layer 2 transforms before aggregating
(z = h @ W_l2, 256->128) so both gathers are 128-wide.  One AllGather of z
between the layers; weights replicated.
"""

import numpy as np

N = 50000
E = 800000
D = 128
H = 256
M = 8
NS = N // M          # 6250 nodes per shard
WIN = (NS + 127) // 128   # 49 windows of 128 node slots
NSP = WIN * 128      # 6272 padded shard size
SQRT_HALF = 0.7071067811865476

_CACHE = {}


def _build(T_w):
    import concourse.bacc as bacc
    import concourse.tile as tile
    from concourse import bass, mybir
    from contextlib import ExitStack

    f32 = mybir.dt.float32
    i32 = mybir.dt.int32
    AF = mybir.ActivationFunctionType
    OP = mybir.AluOpType
    T = WIN * T_w

    nc = bacc.Bacc("TRN2", target_bir_lowering=False, debug=False)

    x_ext = nc.dram_tensor("xfull", [N, D], f32, kind="ExternalInput")
    xT_ext = nc.dram_tensor("xT", [128, NSP], f32, kind="ExternalInput")
    esrc_ext = nc.dram_tensor("esrc", [128, T], i32, kind="ExternalInput")
    esrc2_ext = nc.dram_tensor("esrc2", [128, T], i32, kind="ExternalInput")
    erel_ext = nc.dram_tensor("erel", [128, T], f32, kind="ExternalInput")
    ew_ext = nc.dram_tensor("ew", [128, T], f32, kind="ExternalInput")
    wl1_ext = nc.dram_tensor("wl1", [128, 256], f32, kind="ExternalInput")
    wr1_ext = nc.dram_tensor("wr1", [128, 256], f32, kind="ExternalInput")
    wl2_ext = nc.dram_tensor("wl2", [256, 128], f32, kind="ExternalInput")
    wr2_ext = nc.dram_tensor("wr2", [256, 128], f32, kind="ExternalInput")
    b1_ext = nc.dram_tensor("b1c", [128, 2], f32, kind="ExternalInput")
    b2_ext = nc.dram_tensor("b2b", [128, 128], f32, kind="ExternalInput")
    jc_ext = nc.dram_tensor("jc", [128, 128], f32, kind="ExternalInput")
    out_ext = nc.dram_tensor("out", [NS, D], f32, kind="ExternalOutput")

    with tile.TileContext(nc) as tc, ExitStack() as ctx:
        const = ctx.enter_context(tc.tile_pool(name="const", bufs=1))
        meta = ctx.enter_context(tc.tile_pool(name="meta", bufs=1))
        hpool = ctx.enter_context(tc.tile_pool(name="hpool", bufs=1))
        gbuf = ctx.enter_context(tc.tile_pool(name="gbuf", bufs=8))
        spool = ctx.enter_context(tc.tile_pool(name="spool", bufs=6))
        work = ctx.enter_context(tc.tile_pool(name="work", bufs=2))
        pag = ctx.enter_context(tc.tile_pool(name="pag", bufs=2, space="PSUM"))
        ph = ctx.enter_context(tc.tile_pool(name="ph", bufs=2, space="PSUM"))
        pz = ctx.enter_context(tc.tile_pool(name="pz", bufs=2, space="PSUM"))
        po = ctx.enter_context(tc.tile_pool(name="po", bufs=2, space="PSUM"))
        dram = ctx.enter_context(tc.tile_pool(name="dram", bufs=1, space="DRAM"))

        def load(pool, shape, dt, src, nm):
            t = pool.tile(shape, dt, name=nm)
            nc.sync.dma_start(t[:], src)
            return t

        wl1_t = load(const, [128, 256], f32, wl1_ext[:], "ld_wl1")
        wr1_t = load(const, [128, 256], f32, wr1_ext[:], "ld_wr1")
        wl2a_t = load(const, [128, 128], f32, wl2_ext[0:128, :], "ld_wl2a")
        wl2b_t = load(const, [128, 128], f32, wl2_ext[128:256, :], "ld_wl2b")
        wr2a_t = load(const, [128, 128], f32, wr2_ext[0:128, :], "ld_wr2a")
        wr2b_t = load(const, [128, 128], f32, wr2_ext[128:256, :], "ld_wr2b")
        b1_t = load(const, [128, 2], f32, b1_ext[:], "ld_b1")
        b2_t = load(const, [128, 128], f32, b2_ext[:], "ld_b2")
        jc_t = load(const, [128, 128], f32, jc_ext[:], "ld_jc")
        xT_t = load(meta, [128, NSP], f32, xT_ext[:], "ld_xT")
        esrc_t = load(meta, [128, T], i32, esrc_ext[:], "ld_esrc")
        esrc2_t = load(meta, [128, T], i32, esrc2_ext[:], "ld_esrc2")
        erel_t = load(meta, [128, T], f32, erel_ext[:], "ld_erel")
        ew_t = load(meta, [128, T], f32, ew_ext[:], "ld_ew")

        hT0 = hpool.tile([128, NSP], f32, name="hT0")
        hT1 = hpool.tile([128, NSP], f32, name="hT1")
        z_local = dram.tile([NSP, D], f32, name="z_local")
        z_full = dram.tile([M * NSP, D], f32, name="z_full", addr_space="Shared")

        def build_s(col):
            s = spool.tile([128, 128], f32, name="s")
            nc.vector.tensor_scalar(
                s[:], jc_t[:],
                erel_t[:, col:col + 1], ew_t[:, col:col + 1],
                OP.is_equal, OP.mult,
            )
            return s

        # ---------------- Layer 1 ----------------
        for w in range(WIN):
            cs, ce = w * 128, (w + 1) * 128
            p_agg = pag.tile([128, 128], f32, name="p_agg")
            for k in range(T_w):
                col = w * T_w + k
                xg = gbuf.tile([128, D], f32, name="xg")
                nc.gpsimd.indirect_dma_start(
                    out=xg[:], out_offset=None, in_=x_ext[:],
                    in_offset=bass.IndirectOffsetOnAxis(
                        ap=esrc_t[:, col:col + 1], axis=0),
                )
                s = build_s(col)
                nc.tensor.matmul(
                    out=p_agg[:], lhsT=xg[:], rhs=s[:],
                    start=(k == 0), stop=(k == T_w - 1),
                )
            aggT = work.tile([128, 128], f32, name="aggT")
            nc.vector.tensor_copy(aggT[:], p_agg[:])
            for j in range(2):
                p_h = ph.tile([128, 128], f32, name="p_h")
                nc.tensor.matmul(
                    out=p_h[:], lhsT=wl1_t[:, j * 128:(j + 1) * 128], rhs=aggT[:],
                    start=True, stop=False)
                nc.tensor.matmul(
                    out=p_h[:], lhsT=wr1_t[:, j * 128:(j + 1) * 128],
                    rhs=xT_t[:, cs:ce], start=False, stop=True)
                # exact GELU, stored unscaled: h = u * (1 + erf(u/sqrt(2)))
                # (the 0.5 is folded into W_l2/W_r2 on the host)
                u = work.tile([128, 128], f32, name="u")
                nc.scalar.activation(u[:], p_h[:], AF.Identity, bias=b1_t[:, j:j + 1])
                t_ = work.tile([128, 128], f32, name="t_")
                nc.scalar.activation(t_[:], u[:], AF.Erf, scale=SQRT_HALF)
                v = work.tile([128, 128], f32, name="v")
                nc.vector.tensor_tensor(v[:], u[:], t_[:], op=OP.mult)
                hT = hT0 if j == 0 else hT1
                nc.vector.tensor_tensor(hT[:, cs:ce], u[:], v[:], op=OP.add)
            p_z = pz.tile([128, 128], f32, name="p_z")
            nc.tensor.matmul(out=p_z[:], lhsT=hT0[:, cs:ce], rhs=wl2a_t[:],
                             start=True, stop=False)
            nc.tensor.matmul(out=p_z[:], lhsT=hT1[:, cs:ce], rhs=wl2b_t[:],
                             start=False, stop=True)
            zt = work.tile([128, 128], f32, name="zt")
            nc.scalar.activation(zt[:], p_z[:], AF.Copy)
            nc.sync.dma_start(z_local[cs:ce, :], zt[:])

        nc.gpsimd.collective_compute(
            "AllGather",
            mybir.AluOpType.bypass,
            replica_groups=[list(range(M))],
            ins=[z_local.opt()],
            outs=[z_full.opt()],
        )

        # ---------------- Layer 2 ----------------
        for w in range(WIN):
            cs, ce = w * 128, (w + 1) * 128
            p_o = po.tile([128, 128], f32, name="p_o")
            for k in range(T_w):
                col = w * T_w + k
                zg = gbuf.tile([128, D], f32, name="zg")
                nc.gpsimd.indirect_dma_start(
                    out=zg[:], out_offset=None, in_=z_full,
                    in_offset=bass.IndirectOffsetOnAxis(
                        ap=esrc2_t[:, col:col + 1], axis=0),
                )
                s = build_s(col)
                nc.tensor.matmul(
                    out=p_o[:], lhsT=s[:], rhs=zg[:],
                    start=(k == 0), stop=False,
                )
            nc.tensor.matmul(out=p_o[:], lhsT=hT0[:, cs:ce], rhs=wr2a_t[:],
                             start=False, stop=False)
            nc.tensor.matmul(out=p_o[:], lhsT=hT1[:, cs:ce], rhs=wr2b_t[:],
                             start=False, stop=True)
            ot = work.tile([128, 128], f32, name="ot")
            nc.vector.tensor_tensor(ot[:], p_o[:], b2_t[:], op=OP.add)
            rows = min(128, NS - w * 128)
            nc.sync.dma_start(out_ext[w * 128:w * 128 + rows, :], ot[:rows, :])

    nc.compile()
    return nc


def _host_prep(x, edge_index, W_l1, W_r1, b1, W_l2, W_r2, b2):
    x = np.ascontiguousarray(np.asarray(x, np.float32))
    ei = np.asarray(edge_index, np.int64)
    src, dst = ei[0], ei[1]

    cnt = np.bincount(dst, minlength=N).astype(np.float32)
    inv = 1.0 / np.maximum(cnt, 1.0)

    order = np.argsort(dst, kind="stable")
    s_src = src[order]
    s_dst = dst[order]
    s_shard = s_dst // NS
    s_loc = s_dst - s_shard * NS
    s_win = s_loc // 128
    s_rel = (s_loc % 128).astype(np.float32)
    gwin = s_shard * WIN + s_win
    counts = np.bincount(gwin, minlength=M * WIN)
    T_w = max(1, int(np.ceil(counts.max() / 128)))
    T = WIN * T_w

    gstart = np.concatenate([[0], np.cumsum(counts)[:-1]])
    pos = np.arange(E) - gstart[gwin]
    part = pos % 128
    col = s_win * T_w + pos // 128

    esrc = np.zeros((M, 128, T), np.int32)
    esrc2 = np.zeros((M, 128, T), np.int32)
    erel = np.full((M, 128, T), -1.0, np.float32)
    ew = np.zeros((M, 128, T), np.float32)
    esrc[s_shard, part, col] = s_src
    src_shard = s_src // NS
    esrc2[s_shard, part, col] = src_shard * NSP + (s_src - src_shard * NS)
    erel[s_shard, part, col] = s_rel
    ew[s_shard, part, col] = inv[s_dst]

    xT = np.zeros((M, 128, NSP), np.float32)
    for c in range(M):
        xT[c, :, :NS] = x[c * NS:(c + 1) * NS].T

    W_l1 = np.ascontiguousarray(np.asarray(W_l1, np.float32))
    W_r1 = np.ascontiguousarray(np.asarray(W_r1, np.float32))
    wl2 = np.ascontiguousarray(0.5 * np.asarray(W_l2, np.float32))
    wr2 = np.ascontiguousarray(0.5 * np.asarray(W_r2, np.float32))
    b1 = np.asarray(b1, np.float32)
    b1c = np.ascontiguousarray(np.stack([b1[:128], b1[128:]], axis=1))
    b2b = np.ascontiguousarray(
        np.tile(np.asarray(b2, np.float32)[None, :], (128, 1)))
    jc = np.ascontiguousarray(
        np.tile(np.arange(128, dtype=np.float32)[None, :], (128, 1)))

    in_maps = []
    for c in range(M):
        in_maps.append({
            "xfull": x,
            "xT": np.ascontiguousarray(xT[c]),
            "esrc": np.ascontiguousarray(esrc[c]),
            "esrc2": np.ascontiguousarray(esrc2[c]),
            "erel": np.ascontiguousarray(erel[c]),
            "ew": np.ascontiguousarray(ew[c]),
            "wl1": W_l1,
            "wr1": W_r1,
            "wl2": wl2,
            "wr2": wr2,
            "b1c": b1c,
            "b2b": b2b,
            "jc": jc,
        })
    return in_maps, T_w


def kernel(x, edge_index, W_l1, W_r1, b1, W_l2, W_r2, b2, _trace=False):
    from concourse import bass_utils

    in_maps, T_w = _host_prep(x, edge_index, W_l1, W_r1, b1, W_l2, W_r2, b2)
    if T_w not in _CACHE:
        _CACHE[T_w] = _build(T_w)
    nc = _CACHE[T_w]
    res = bass_utils.run_bass_kernel_spmd(
        nc, in_maps, core_ids=list(range(M)), trace=_trace)
    out = np.concatenate([res.results[c]["out"] for c in range(M)], axis=0)
    if _trace:
        kernel.last_exec_time_ns = res.exec_time_ns
        kernel.last_results = res
    return out

